# revision 27
# baseline (speedup 1.0000x reference)
"""Distributed sparse MoE (top-1 routing) kernel for 8 TRN2 NeuronCores.

Strategy (zero-collective data-parallel):
  - Core c owns token slice [c*1024, (c+1)*1024) and ALL 8 expert weights
    (host-replicated bf16). No collectives -> core 0 never waits on the
    launch skew of its peers.
  - The sync queue is a dedicated streaming FIFO (HWDGE DMAs occupy their
    issuing engine for the whole transfer): router xT quarters first, then
    the 8 x 2MB expert weights (SBUF-resident), then the staged output
    writes. Small latency-critical DMAs go on the scalar queue.
  - Router: fp32 PE matmul in stream orientation (lhsT = router_w chunk,
    rhs = xT chunk) -> logitsT [8, 1024] in PSUM, bias via per-partition
    scalar add at eviction, then 8 small PE transposes give [128, 8]
    logit tiles (argmax must match the reference bit-for-bit: min top-2
    logit gap ~4e-5, so routing stays fp32 while expert GEMMs are bf16).
    Batched softmax: E=exp(logits) (|logit| <~ 6), per-8-group sum/max
    reductions, gate = max(E)/sum(E).
  - Selection packs id and gate into ONE value per token:
    v = (tokid + gate/2) after masking, compacted per expert by
    sparse_gather (capacity 256/expert). Compaction results stream to
    DRAM per expert and come back in ONE re-wrap read; on-chip unpack
    gives id = trunc(v), gate = 2*(v - trunc(v)). Pad slots are forced
    to the OOB sentinel AFTER the roundtrip (select, NaN-safe) using a
    count broadcast that overlaps the roundtrip itself.
  - Per expert: indirect-gather token rows (bf16; the SWDGE queue holds
    ONLY gathers), PE-transpose, bf16 GEMM vs resident W_e with fp32
    accumulate, bias + gate at PSUM eviction. Outputs are written
    CONTIGUOUSLY (direct DMA, line rate) as staged [2048, H] bf16 plus
    the 8KB permutation tensor; the host unpermutes (slot -> token row)
    while concatenating the 8 disjoint slice outputs and casting f32.
"""

import sys

sys.path.insert(0, "/opt/trn_rl_repo")

import ml_dtypes
import numpy as np

import concourse.bass as bass
import concourse.mybir as mybir
import concourse.tile as tile
from concourse import bacc
from concourse.bass_utils import run_bass_kernel_spmd
from concourse.masks import make_identity

F32 = mybir.dt.float32
BF16 = mybir.dt.bfloat16
I32 = mybir.dt.int32
U32 = mybir.dt.uint32

N_CORES = 8
B, S, H, E = 4, 2048, 1024, 8
T = B * S                # 8192 tokens
TPC = T // N_CORES       # 1024 tokens per core slice
TILES = TPC // 128       # 8 token tiles per slice
HC = H // 128            # 8 contraction chunks
CAPZ = 256               # per-(core,expert) token capacity (mean 128, sigma ~11)
ZTIL = CAPZ // 128       # 2 gathered token tiles per expert
NHALF = 2                # 1024 output dims in 2 x 512 psum halves
OOB = TPC                # out-of-bounds sentinel id (skipped / host-dropped)
SEL = TILES * E          # 64: free size of the [16, .] selection layout
NGT = E * ZTIL           # 16 gather tiles
PREF = 5                 # gather prefetch depth


def _body(tc, xt, xb, rw, rb, ew, eb, iota1, out, perm, cnts):
    nc = tc.nc
    P = 128
    Exp = mybir.ActivationFunctionType.Exp

    const = tc.alloc_tile_pool(name="const", bufs=1)

    # --- streaming FIFO (sync queue): xt chunks first, then weights ---
    xtp = tc.alloc_tile_pool(name="xtp", bufs=3)
    xcs = []
    for c in range(HC):
        xc = xtp.tile([P, TPC], F32, tag="xc")
        nc.sync.dma_start(xc[:], xt[c * P : (c + 1) * P, :])
        xcs.append(xc)
    w_sb = []
    for e in range(E):
        wt = const.tile([P, HC, H], BF16, name=f"w{e}")
        nc.sync.dma_start(
            wt[:], ew[e * H : (e + 1) * H, :].rearrange("(c p) d -> p c d", p=P)
        )
        w_sb.append(wt)

    # --- small constants (scalar queue) ---
    rw_sb = const.tile([P, HC, E], F32)
    nc.scalar.dma_start(rw_sb[:], rw.rearrange("(c p) e -> p c e", p=P))
    rb_sb = const.tile([E, 1], F32)
    nc.scalar.dma_start(rb_sb[:], rb[:])
    ident = const.tile([P, P], F32)
    make_identity(nc, ident)
    identb = const.tile([P, P], BF16)
    nc.vector.tensor_copy(identb[:], ident[:])
    iota_sb = const.tile([16, SEL], F32)
    nc.scalar.dma_start(iota_sb[:], iota1[:])

    dram = tc.alloc_tile_pool(name="dram", bufs=1, space="DRAM")
    dec_dram = dram.tile([P, 16], F32)
    ig_dram = dram.tile([E, CAPZ], F32)

    # ---- Phase A: router, stream orientation ----
    dec_sb = const.tile([P, 16], F32)
    lT_sb = const.tile([8, TPC], F32)
    logits = const.tile([P, TILES, E], F32)
    with tc.tile_pool(name="workA", bufs=2) as workA, tc.tile_pool(
        name="psumL", bufs=1, space="PSUM"
    ) as psumL, tc.tile_pool(name="psumR", bufs=1, space="PSUM") as psumR:
        lpT = psumL.tile([8, TPC], F32)
        for c in range(HC):
            for h in range(NHALF):
                nc.tensor.matmul(
                    lpT[:, h * 512 : (h + 1) * 512],
                    lhsT=rw_sb[:, c, :],
                    rhs=xcs[c][:, h * 512 : (h + 1) * 512],
                    start=(c == 0),
                    stop=(c == HC - 1),
                )
        # evict with router bias (per-partition scalar), then transpose
        nc.vector.tensor_scalar(
            lT_sb[:], lpT[:], rb_sb[:], None, op0=mybir.AluOpType.add
        )
        ptil = psumR.tile([P, TILES, E], F32)
        for t in range(TILES):
            nc.tensor.transpose(
                ptil[:, t, :], lT_sb[:, t * P : (t + 1) * P], ident[0:8, 0:8]
            )
        nc.vector.tensor_copy(
            logits[:].rearrange("p a b -> p (a b)"),
            ptil[:].rearrange("p a b -> p (a b)"),
        )
        # batched softmax pieces: exp, per-8-group sum and max
        expd = workA.tile([P, TILES, E], F32, tag="expd")
        nc.scalar.activation(
            expd[:].rearrange("p a b -> p (a b)"),
            logits[:].rearrange("p a b -> p (a b)"),
            Exp,
        )
        esum = workA.tile([P, TILES], F32, tag="esum")
        nc.vector.reduce_sum(esum[:], expd[:], mybir.AxisListType.X)
        emax = workA.tile([P, TILES], F32, tag="emax")
        nc.vector.reduce_max(emax[:], expd[:], mybir.AxisListType.X)
        erec = workA.tile([P, TILES], F32, tag="erec")
        nc.vector.reciprocal(erec[:], esum[:])
        nc.vector.tensor_tensor(
            dec_sb[:, 8:16], emax[:], erec[:], mybir.AluOpType.mult
        )
        for t in range(TILES):
            mx8 = workA.tile([P, 8], F32, tag="mx8")
            nc.vector.max(mx8[:], logits[:, t, :])
            mi = workA.tile([P, 8], U32, tag="mi")
            nc.vector.max_index(mi[:], mx8[:], logits[:, t, :])
            nc.vector.tensor_copy(dec_sb[:, t : t + 1], mi[:, 0:1])
    xtp.release()

    # ---- Phase B: selection — single-engine (gpsimd) chain, no cross-
    # engine ping-pong. The compaction output is pre-filled with the OOB
    # sentinel; sparse_gather only overwrites the slots it found, so tail
    # slots need no fixup (id -> 1024 skipped by bounds check / host).
    sel = tc.alloc_tile_pool(name="sel", bufs=1)
    stage_all = sel.tile([16, E, CAPZ // 16], F32)
    nc.gpsimd.memset(stage_all[:].rearrange("p a b -> p (a b)"), float(OOB))
    # roundtrip through DRAM to re-wrap [128,16] -> [16,128]
    nc.gpsimd.dma_start(dec_dram[:], dec_sb[:])
    dsb = sel.tile([16, 8, 16], F32)
    nc.gpsimd.dma_start(dsb[:], dec_dram[:].rearrange("(p a) c -> p a c", p=16))
    idx16 = sel.tile([16, SEL], F32)
    nc.gpsimd.tensor_copy(idx16[:].rearrange("p (a b) -> p a b", a=8), dsb[:, :, 0:8])
    # packed compaction value: base = (tokid+1) + gate/2; the -1 of the
    # masking below shifts it to tokid + gate/2 for selected slots
    base = sel.tile([16, SEL], F32)
    nc.gpsimd.tensor_scalar(
        base[:].rearrange("p (a b) -> p a b", a=8),
        dsb[:, :, 8:16],
        0.5,
        None,
        op0=mybir.AluOpType.mult,
    )
    nc.gpsimd.tensor_tensor(base[:], base[:], iota_sb[:], mybir.AluOpType.add)
    val_all = sel.tile([16, E, SEL], F32)
    for e in range(E):
        eqv = val_all[:, e, :]
        nc.gpsimd.tensor_scalar(
            eqv, idx16[:], float(e), None, op0=mybir.AluOpType.is_equal
        )
        nc.gpsimd.tensor_tensor(eqv, base[:], eqv, mybir.AluOpType.mult)
        nc.gpsimd.tensor_scalar_add(eqv, eqv, -1.0)
    cnt_all = sel.tile([1, E], U32)
    for e in range(E):
        nc.gpsimd.sparse_gather(
            stage_all[:, e, :], val_all[:, e, :], num_found=cnt_all[:, e : e + 1]
        )
    nc.gpsimd.dma_start(
        ig_dram[:].rearrange("e (f p) -> p e f", p=16), stage_all[:]
    )
    # ONE re-wrap read: [128, (e j)] per-partition slots, then unpack
    igp = sel.tile([P, NGT], F32)
    nc.gpsimd.dma_start(
        igp[:].rearrange("p (e j) -> p e j", e=E),
        ig_dram[:].rearrange("e (j p) -> p e j", p=P),
    )
    idsel = sel.tile([P, NGT], I32)
    nc.gpsimd.tensor_copy(idsel[:], igp[:])         # trunc to tokid
    # compaction tails hold garbage: clamp ids into [0, OOB] (int ops are
    # NaN-free; host drops slots >= count, bounds check skips id == OOB)
    nc.gpsimd.tensor_scalar(idsel[:], idsel[:], 0, None, op0=mybir.AluOpType.max)
    nc.gpsimd.tensor_scalar(idsel[:], idsel[:], OOB, None, op0=mybir.AluOpType.min)
    idxf = sel.tile([P, NGT], F32)
    nc.gpsimd.tensor_copy(idxf[:], idsel[:])
    gativ = sel.tile([P, NGT], F32)
    nc.gpsimd.tensor_tensor(gativ[:], igp[:], idxf[:], mybir.AluOpType.subtract)
    nc.gpsimd.tensor_scalar(gativ[:], gativ[:], 2.0, None, op0=mybir.AluOpType.mult)

    # ---- Phase C per expert: gather, transpose, GEMM, staged write ----
    with tc.tile_pool(name="ebp", bufs=2) as ebp, tc.tile_pool(
        name="workD", bufs=2
    ) as workD, tc.tile_pool(name="gathp", bufs=PREF) as gathp, tc.tile_pool(
        name="outp", bufs=3
    ) as outp, tc.tile_pool(name="psumT", bufs=3, space="PSUM") as psumT, tc.tile_pool(
        name="psumG", bufs=2, space="PSUM"
    ) as psumG:
        gtiles = {}

        def issue_gather(g):
            gt = gathp.tile([P, H], BF16, tag="gath")
            nc.gpsimd.indirect_dma_start(
                out=gt[:],
                out_offset=None,
                in_=xb[:],
                in_offset=bass.IndirectOffsetOnAxis(ap=idsel[:, g : g + 1], axis=0),
                bounds_check=TPC - 1,
                oob_is_err=False,
            )
            gtiles[g] = gt

        for g in range(PREF):
            issue_gather(g)
        for e in range(E):
            eb_sb = ebp.tile([1, H], F32, tag="eb_sb")
            nc.scalar.dma_start(eb_sb[:], eb[e : e + 1, :])
            b_rep = ebp.tile([P, H], F32, tag="b_rep")
            nc.gpsimd.partition_broadcast(b_rep[:], eb_sb[:])
            for j in range(ZTIL):
                g = e * ZTIL + j
                gath = gtiles.pop(g)
                xTg = workD.tile([P, HC, P], BF16, tag="xTg")
                pt = psumT.tile([P, H], BF16, tag="pt")
                for c in range(HC):
                    nc.tensor.transpose(
                        pt[:, c * P : (c + 1) * P],
                        gath[:, c * P : (c + 1) * P],
                        identb[:],
                    )
                if j % 2 == 0:
                    nc.scalar.copy(xTg[:].rearrange("p c d -> p (c d)"), pt[:])
                else:
                    nc.vector.tensor_copy(
                        xTg[:].rearrange("p c d -> p (c d)"), pt[:]
                    )
                outsb = outp.tile([P, H], BF16, tag="outsb")
                for h in range(NHALF):
                    pg = psumG.tile([P, 512], F32, tag="pg")
                    for c in range(HC):
                        nc.tensor.matmul(
                            pg[:],
                            lhsT=xTg[:, c, :],
                            rhs=w_sb[e][:, c, h * 512 : (h + 1) * 512],
                            start=(c == 0),
                            stop=(c == HC - 1),
                        )
                    nc.vector.tensor_tensor(
                        outsb[:, h * 512 : (h + 1) * 512],
                        pg[:],
                        b_rep[:, h * 512 : (h + 1) * 512],
                        mybir.AluOpType.add,
                    )
                    nc.vector.tensor_scalar_mul(
                        outsb[:, h * 512 : (h + 1) * 512],
                        outsb[:, h * 512 : (h + 1) * 512],
                        gativ[:, g : g + 1],
                    )
                nc.sync.dma_start(out[g * P : (g + 1) * P, :], outsb[:])
                if g + PREF < NGT:
                    issue_gather(g + PREF)
    nc.scalar.dma_start(perm[:], idsel[:])
    nc.scalar.dma_start(cnts[:], cnt_all[:])

    sel.release()
    dram.release()
    const.release()


def build_kernel():
    nc = bacc.Bacc(
        "TRN2",
        target_bir_lowering=False,
        debug=False,
        enable_asserts=True,
        num_devices=N_CORES,
    )
    xt = nc.dram_tensor("xt", [H, TPC], F32, kind="ExternalInput").ap()
    xb = nc.dram_tensor("xb", [TPC, H], BF16, kind="ExternalInput").ap()
    rw = nc.dram_tensor("router_w", [H, E], F32, kind="ExternalInput").ap()
    rb = nc.dram_tensor("router_b", [E, 1], F32, kind="ExternalInput").ap()
    ew = nc.dram_tensor("expert_w", [E * H, H], BF16, kind="ExternalInput").ap()
    eb = nc.dram_tensor("expert_b", [E, H], F32, kind="ExternalInput").ap()
    iota1 = nc.dram_tensor("iota1", [16, TILES * E], F32, kind="ExternalInput").ap()
    out = nc.dram_tensor("out", [E * CAPZ, H], BF16, kind="ExternalOutput").ap()
    perm = nc.dram_tensor("perm", [P128, NGT], I32, kind="ExternalOutput").ap()
    cnts = nc.dram_tensor("cnts", [1, E], U32, kind="ExternalOutput").ap()

    with tile.TileContext(nc) as tc:
        _body(tc, xt, xb, rw, rb, ew, eb, iota1, out, perm, cnts)
    nc.compile()
    return nc


P128 = 128
_CACHE = {}


def kernel(x, router_w, router_b, expert_w, expert_b, **run_kwargs):
    x = np.ascontiguousarray(np.asarray(x, dtype=np.float32))
    router_w = np.ascontiguousarray(np.asarray(router_w, dtype=np.float32))
    router_b = np.ascontiguousarray(np.asarray(router_b, dtype=np.float32))
    expert_w = np.ascontiguousarray(np.asarray(expert_w, dtype=np.float32))
    expert_b = np.ascontiguousarray(np.asarray(expert_b, dtype=np.float32))

    hs = x.reshape(T, H)
    ew_b = np.ascontiguousarray(
        expert_w.reshape(E * H, H).astype(ml_dtypes.bfloat16)
    )
    eb_r = np.ascontiguousarray(expert_b.reshape(E, H))
    rb_r = np.ascontiguousarray(router_b.reshape(E, 1))

    # iota1[p, j2]: local token id + 1 at selection position (p, j2)
    # j2 = jj*8 + col; token k = col*128 + 8*p + jj
    pp, j2 = np.meshgrid(np.arange(16), np.arange(TILES * E), indexing="ij")
    jj, col = j2 // 8, j2 % 8
    iota1 = (col * 128 + 8 * pp + jj + 1).astype(np.float32)

    if "nc" not in _CACHE:
        _CACHE["nc"] = build_kernel()
    nc = _CACHE["nc"]

    in_maps = []
    for c in range(N_CORES):
        sl = hs[c * TPC : (c + 1) * TPC]
        in_maps.append(
            {
                "xt": np.ascontiguousarray(sl.T),
                "xb": np.ascontiguousarray(sl.astype(ml_dtypes.bfloat16)),
                "router_w": router_w,
                "router_b": rb_r,
                "expert_w": ew_b,
                "expert_b": eb_r,
                "iota1": iota1,
            }
        )

    res = run_bass_kernel_spmd(nc, in_maps, core_ids=list(range(N_CORES)), **run_kwargs)
    full = np.empty((T, H), dtype=np.float32)
    for c, r in enumerate(res.results):
        staged = np.asarray(r["out"], dtype=np.float32)     # [E*CAPZ, H]
        permv = np.asarray(r["perm"])                       # [128, NGT] i32
        cnt = np.asarray(r["cnts"]).reshape(E).astype(np.int64)
        # slot (p, g) -> staged row g*128+p holds token permv[p, g]
        ids = permv.T.reshape(-1)                           # row-major g*128+p
        slot_in_e = np.tile(np.arange(CAPZ), E)
        valid = (ids < TPC) & (slot_in_e < cnt.repeat(CAPZ))
        sl_out = full[c * TPC : (c + 1) * TPC]
        sl_out[ids[valid]] = staged[valid]
    out = full.reshape(B, S, H)
    if run_kwargs:
        return out, res
    return out


# revision 28
# speedup vs baseline: 1.0146x; 1.0146x over previous
"""Distributed sparse MoE (top-1 routing) kernel for 8 TRN2 NeuronCores.

Strategy (zero-collective data-parallel):
  - Core c owns token slice [c*1024, (c+1)*1024) and ALL 8 expert weights
    (host-replicated bf16). No collectives -> core 0 never waits on the
    launch skew of its peers.
  - The sync queue is a dedicated streaming FIFO (HWDGE DMAs occupy their
    issuing engine for the whole transfer): router xT quarters first, then
    the 8 x 2MB expert weights (SBUF-resident), then the staged output
    writes. Small latency-critical DMAs go on the scalar queue.
  - Router: fp32 PE matmul in stream orientation (lhsT = router_w chunk,
    rhs = xT chunk) -> logitsT [8, 1024] in PSUM, bias via per-partition
    scalar add at eviction, then 8 small PE transposes give [128, 8]
    logit tiles (argmax must match the reference bit-for-bit: min top-2
    logit gap ~4e-5, so routing stays fp32 while expert GEMMs are bf16).
    Batched softmax: E=exp(logits) (|logit| <~ 6), per-8-group sum/max
    reductions, gate = max(E)/sum(E).
  - Selection packs id and gate into ONE value per token:
    v = (tokid + gate/2) after masking, compacted per expert by
    sparse_gather (capacity 256/expert). Compaction results stream to
    DRAM per expert and come back in ONE re-wrap read; on-chip unpack
    gives id = trunc(v), gate = 2*(v - trunc(v)). Pad slots are forced
    to the OOB sentinel AFTER the roundtrip (select, NaN-safe) using a
    count broadcast that overlaps the roundtrip itself.
  - Per expert: indirect-gather token rows (bf16; the SWDGE queue holds
    ONLY gathers), PE-transpose, bf16 GEMM vs resident W_e with fp32
    accumulate, bias + gate at PSUM eviction. Outputs are written
    CONTIGUOUSLY (direct DMA, line rate) as staged [2048, H] bf16 plus
    the 8KB permutation tensor; the host unpermutes (slot -> token row)
    while concatenating the 8 disjoint slice outputs and casting f32.
"""

import sys

sys.path.insert(0, "/opt/trn_rl_repo")

import ml_dtypes
import numpy as np

import concourse.bass as bass
import concourse.mybir as mybir
import concourse.tile as tile
from concourse import bacc
from concourse.bass_utils import run_bass_kernel_spmd
from concourse.masks import make_identity

F32 = mybir.dt.float32
BF16 = mybir.dt.bfloat16
I32 = mybir.dt.int32
U32 = mybir.dt.uint32

N_CORES = 8
B, S, H, E = 4, 2048, 1024, 8
T = B * S                # 8192 tokens
TPC = T // N_CORES       # 1024 tokens per core slice
TILES = TPC // 128       # 8 token tiles per slice
HC = H // 128            # 8 contraction chunks
CAPZ = 256               # per-(core,expert) token capacity (mean 128, sigma ~11)
ZTIL = CAPZ // 128       # 2 gathered token tiles per expert
NHALF = 2                # 1024 output dims in 2 x 512 psum halves
OOB = TPC                # out-of-bounds sentinel id (skipped / host-dropped)
SEL = TILES * E          # 64: free size of the [16, .] selection layout
NGT = E * ZTIL           # 16 gather tiles
PREF = 5                 # gather prefetch depth


def _body(tc, xt, xb, rw, rb, ew, eb, iota1, out, perm, cnts):
    nc = tc.nc
    P = 128
    Exp = mybir.ActivationFunctionType.Exp

    const = tc.alloc_tile_pool(name="const", bufs=1)

    # --- streaming FIFO (sync queue): xt chunks first, then weights ---
    xtp = tc.alloc_tile_pool(name="xtp", bufs=3)
    xcs = []
    for c in range(HC):
        xc = xtp.tile([P, TPC], F32, tag="xc")
        nc.sync.dma_start(xc[:], xt[c * P : (c + 1) * P, :])
        xcs.append(xc)
    w_sb = []
    for e in range(E):
        wt = const.tile([P, HC, H], BF16, name=f"w{e}")
        nc.sync.dma_start(
            wt[:], ew[e * H : (e + 1) * H, :].rearrange("(c p) d -> p c d", p=P)
        )
        w_sb.append(wt)

    # --- small constants (scalar queue) ---
    rw_sb = const.tile([P, HC, E], F32)
    nc.scalar.dma_start(rw_sb[:], rw.rearrange("(c p) e -> p c e", p=P))
    rb_sb = const.tile([E, 1], F32)
    nc.scalar.dma_start(rb_sb[:], rb[:])
    ident = const.tile([P, P], F32)
    make_identity(nc, ident)
    identb = const.tile([P, P], BF16)
    nc.vector.tensor_copy(identb[:], ident[:])
    iota_sb = const.tile([16, SEL], F32)
    nc.scalar.dma_start(iota_sb[:], iota1[:])

    dram = tc.alloc_tile_pool(name="dram", bufs=1, space="DRAM")
    dec_dram = dram.tile([P, 16], F32)
    ig_dram = dram.tile([E, CAPZ], F32)

    # ---- Phase A: router, stream orientation ----
    dec_sb = const.tile([P, 16], F32)
    lT_sb = const.tile([8, TPC], F32)
    logits = const.tile([P, TILES, E], F32)
    with tc.tile_pool(name="workA", bufs=2) as workA, tc.tile_pool(
        name="psumL", bufs=1, space="PSUM"
    ) as psumL, tc.tile_pool(name="psumR", bufs=1, space="PSUM") as psumR:
        lpT = psumL.tile([8, TPC], F32)
        for c in range(HC):
            for h in range(NHALF):
                nc.tensor.matmul(
                    lpT[:, h * 512 : (h + 1) * 512],
                    lhsT=rw_sb[:, c, :],
                    rhs=xcs[c][:, h * 512 : (h + 1) * 512],
                    start=(c == 0),
                    stop=(c == HC - 1),
                )
        # evict with router bias (per-partition scalar), then transpose
        nc.vector.tensor_scalar(
            lT_sb[:], lpT[:], rb_sb[:], None, op0=mybir.AluOpType.add
        )
        ptil = psumR.tile([P, TILES, E], F32)
        for t in range(TILES):
            nc.tensor.transpose(
                ptil[:, t, :], lT_sb[:, t * P : (t + 1) * P], ident[0:8, 0:8]
            )
        nc.vector.tensor_copy(
            logits[:].rearrange("p a b -> p (a b)"),
            ptil[:].rearrange("p a b -> p (a b)"),
        )
        # batched softmax pieces: exp, per-8-group sum and max
        expd = workA.tile([P, TILES, E], F32, tag="expd")
        nc.scalar.activation(
            expd[:].rearrange("p a b -> p (a b)"),
            logits[:].rearrange("p a b -> p (a b)"),
            Exp,
        )
        esum = workA.tile([P, TILES], F32, tag="esum")
        nc.vector.reduce_sum(esum[:], expd[:], mybir.AxisListType.X)
        emax = workA.tile([P, TILES], F32, tag="emax")
        nc.vector.reduce_max(emax[:], expd[:], mybir.AxisListType.X)
        erec = workA.tile([P, TILES], F32, tag="erec")
        nc.vector.reciprocal(erec[:], esum[:])
        nc.vector.tensor_tensor(
            dec_sb[:, 8:16], emax[:], erec[:], mybir.AluOpType.mult
        )
        for t in range(TILES):
            mx8 = workA.tile([P, 8], F32, tag="mx8")
            nc.vector.max(mx8[:], logits[:, t, :])
            mi = workA.tile([P, 8], U32, tag="mi")
            nc.vector.max_index(mi[:], mx8[:], logits[:, t, :])
            nc.vector.tensor_copy(dec_sb[:, t : t + 1], mi[:, 0:1])
    xtp.release()

    # ---- Phase B: selection — single-engine (gpsimd) chain, no cross-
    # engine ping-pong. The compaction output is pre-filled with the OOB
    # sentinel; sparse_gather only overwrites the slots it found, so tail
    # slots need no fixup (id -> 1024 skipped by bounds check / host).
    sel = tc.alloc_tile_pool(name="sel", bufs=1)
    stage_all = sel.tile([16, E, CAPZ // 16], F32)
    # roundtrip through DRAM to re-wrap [128,16] -> [16,128]
    nc.gpsimd.dma_start(dec_dram[:], dec_sb[:])
    dsb = sel.tile([16, 8, 16], F32)
    nc.gpsimd.dma_start(dsb[:], dec_dram[:].rearrange("(p a) c -> p a c", p=16))
    idx16 = sel.tile([16, SEL], F32)
    nc.vector.tensor_copy(idx16[:].rearrange("p (a b) -> p a b", a=8), dsb[:, :, 0:8])
    # packed compaction value: base = (tokid+1) + gate/2; the -1 of the
    # masking below shifts it to tokid + gate/2 for selected slots
    base = sel.tile([16, SEL], F32)
    nc.vector.tensor_scalar(
        base[:].rearrange("p (a b) -> p a b", a=8),
        dsb[:, :, 8:16],
        0.5,
        None,
        op0=mybir.AluOpType.mult,
    )
    nc.vector.tensor_tensor(base[:], base[:], iota_sb[:], mybir.AluOpType.add)
    val_all = sel.tile([16, E, SEL], F32)
    for e in range(E):
        eqv = val_all[:, e, :]
        nc.vector.tensor_scalar(
            eqv, idx16[:], float(e), None, op0=mybir.AluOpType.is_equal
        )
        nc.vector.tensor_tensor(eqv, base[:], eqv, mybir.AluOpType.mult)
        nc.vector.tensor_scalar_add(eqv, eqv, -1.0)
    cnt_all = sel.tile([1, E], U32)
    for e in range(E):
        nc.gpsimd.sparse_gather(
            stage_all[:, e, :], val_all[:, e, :], num_found=cnt_all[:, e : e + 1]
        )
    nc.gpsimd.dma_start(
        ig_dram[:].rearrange("e (f p) -> p e f", p=16), stage_all[:]
    )
    # ONE re-wrap read: [128, (e j)] per-partition slots, then unpack
    igp = sel.tile([P, NGT], F32)
    nc.gpsimd.dma_start(
        igp[:].rearrange("p (e j) -> p e j", e=E),
        ig_dram[:].rearrange("e (j p) -> p e j", p=P),
    )
    idsel = sel.tile([P, NGT], I32)
    nc.gpsimd.tensor_copy(idsel[:], igp[:])         # trunc to tokid
    # compaction tails hold garbage: clamp ids into [0, OOB] (int ops are
    # NaN-free; host drops slots >= count, bounds check skips id == OOB)
    nc.gpsimd.tensor_scalar(idsel[:], idsel[:], 0, None, op0=mybir.AluOpType.max)
    nc.gpsimd.tensor_scalar(idsel[:], idsel[:], OOB, None, op0=mybir.AluOpType.min)
    idxf = sel.tile([P, NGT], F32)
    nc.gpsimd.tensor_copy(idxf[:], idsel[:])
    gativ = sel.tile([P, NGT], F32)
    nc.gpsimd.tensor_tensor(gativ[:], igp[:], idxf[:], mybir.AluOpType.subtract)
    nc.gpsimd.tensor_scalar(gativ[:], gativ[:], 2.0, None, op0=mybir.AluOpType.mult)

    # ---- Phase C per expert: gather, transpose, GEMM, staged write ----
    with tc.tile_pool(name="ebp", bufs=2) as ebp, tc.tile_pool(
        name="workD", bufs=2
    ) as workD, tc.tile_pool(name="gathp", bufs=PREF) as gathp, tc.tile_pool(
        name="outp", bufs=3
    ) as outp, tc.tile_pool(name="psumT", bufs=3, space="PSUM") as psumT, tc.tile_pool(
        name="psumG", bufs=2, space="PSUM"
    ) as psumG:
        gtiles = {}

        def issue_gather(g):
            gt = gathp.tile([P, H], BF16, tag="gath")
            nc.gpsimd.indirect_dma_start(
                out=gt[:],
                out_offset=None,
                in_=xb[:],
                in_offset=bass.IndirectOffsetOnAxis(ap=idsel[:, g : g + 1], axis=0),
                bounds_check=TPC - 1,
                oob_is_err=False,
            )
            gtiles[g] = gt

        for g in range(PREF):
            issue_gather(g)
        for e in range(E):
            eb_sb = ebp.tile([1, H], F32, tag="eb_sb")
            nc.scalar.dma_start(eb_sb[:], eb[e : e + 1, :])
            b_rep = ebp.tile([P, H], F32, tag="b_rep")
            nc.gpsimd.partition_broadcast(b_rep[:], eb_sb[:])
            for j in range(ZTIL):
                g = e * ZTIL + j
                gath = gtiles.pop(g)
                xTg = workD.tile([P, HC, P], BF16, tag="xTg")
                pt = psumT.tile([P, H], BF16, tag="pt")
                for c in range(HC):
                    nc.tensor.transpose(
                        pt[:, c * P : (c + 1) * P],
                        gath[:, c * P : (c + 1) * P],
                        identb[:],
                    )
                if j % 2 == 0:
                    nc.scalar.copy(xTg[:].rearrange("p c d -> p (c d)"), pt[:])
                else:
                    nc.vector.tensor_copy(
                        xTg[:].rearrange("p c d -> p (c d)"), pt[:]
                    )
                outsb = outp.tile([P, H], BF16, tag="outsb")
                for h in range(NHALF):
                    pg = psumG.tile([P, 512], F32, tag="pg")
                    for c in range(HC):
                        nc.tensor.matmul(
                            pg[:],
                            lhsT=xTg[:, c, :],
                            rhs=w_sb[e][:, c, h * 512 : (h + 1) * 512],
                            start=(c == 0),
                            stop=(c == HC - 1),
                        )
                    nc.vector.tensor_tensor(
                        outsb[:, h * 512 : (h + 1) * 512],
                        pg[:],
                        b_rep[:, h * 512 : (h + 1) * 512],
                        mybir.AluOpType.add,
                    )
                    nc.vector.tensor_scalar_mul(
                        outsb[:, h * 512 : (h + 1) * 512],
                        outsb[:, h * 512 : (h + 1) * 512],
                        gativ[:, g : g + 1],
                    )
                nc.sync.dma_start(out[g * P : (g + 1) * P, :], outsb[:])
                if g + PREF < NGT:
                    issue_gather(g + PREF)
    nc.scalar.dma_start(perm[:], idsel[:])
    nc.scalar.dma_start(cnts[:], cnt_all[:])

    sel.release()
    dram.release()
    const.release()


def build_kernel():
    nc = bacc.Bacc(
        "TRN2",
        target_bir_lowering=False,
        debug=False,
        enable_asserts=True,
        num_devices=N_CORES,
    )
    xt = nc.dram_tensor("xt", [H, TPC], F32, kind="ExternalInput").ap()
    xb = nc.dram_tensor("xb", [TPC, H], BF16, kind="ExternalInput").ap()
    rw = nc.dram_tensor("router_w", [H, E], F32, kind="ExternalInput").ap()
    rb = nc.dram_tensor("router_b", [E, 1], F32, kind="ExternalInput").ap()
    ew = nc.dram_tensor("expert_w", [E * H, H], BF16, kind="ExternalInput").ap()
    eb = nc.dram_tensor("expert_b", [E, H], F32, kind="ExternalInput").ap()
    iota1 = nc.dram_tensor("iota1", [16, TILES * E], F32, kind="ExternalInput").ap()
    out = nc.dram_tensor("out", [E * CAPZ, H], BF16, kind="ExternalOutput").ap()
    perm = nc.dram_tensor("perm", [P128, NGT], I32, kind="ExternalOutput").ap()
    cnts = nc.dram_tensor("cnts", [1, E], U32, kind="ExternalOutput").ap()

    with tile.TileContext(nc) as tc:
        _body(tc, xt, xb, rw, rb, ew, eb, iota1, out, perm, cnts)
    nc.compile()
    return nc


P128 = 128
_CACHE = {}


def kernel(x, router_w, router_b, expert_w, expert_b, **run_kwargs):
    x = np.ascontiguousarray(np.asarray(x, dtype=np.float32))
    router_w = np.ascontiguousarray(np.asarray(router_w, dtype=np.float32))
    router_b = np.ascontiguousarray(np.asarray(router_b, dtype=np.float32))
    expert_w = np.ascontiguousarray(np.asarray(expert_w, dtype=np.float32))
    expert_b = np.ascontiguousarray(np.asarray(expert_b, dtype=np.float32))

    hs = x.reshape(T, H)
    ew_b = np.ascontiguousarray(
        expert_w.reshape(E * H, H).astype(ml_dtypes.bfloat16)
    )
    eb_r = np.ascontiguousarray(expert_b.reshape(E, H))
    rb_r = np.ascontiguousarray(router_b.reshape(E, 1))

    # iota1[p, j2]: local token id + 1 at selection position (p, j2)
    # j2 = jj*8 + col; token k = col*128 + 8*p + jj
    pp, j2 = np.meshgrid(np.arange(16), np.arange(TILES * E), indexing="ij")
    jj, col = j2 // 8, j2 % 8
    iota1 = (col * 128 + 8 * pp + jj + 1).astype(np.float32)

    if "nc" not in _CACHE:
        _CACHE["nc"] = build_kernel()
    nc = _CACHE["nc"]

    in_maps = []
    for c in range(N_CORES):
        sl = hs[c * TPC : (c + 1) * TPC]
        in_maps.append(
            {
                "xt": np.ascontiguousarray(sl.T),
                "xb": np.ascontiguousarray(sl.astype(ml_dtypes.bfloat16)),
                "router_w": router_w,
                "router_b": rb_r,
                "expert_w": ew_b,
                "expert_b": eb_r,
                "iota1": iota1,
            }
        )

    res = run_bass_kernel_spmd(nc, in_maps, core_ids=list(range(N_CORES)), **run_kwargs)
    full = np.empty((T, H), dtype=np.float32)
    for c, r in enumerate(res.results):
        staged = np.asarray(r["out"], dtype=np.float32)     # [E*CAPZ, H]
        permv = np.asarray(r["perm"])                       # [128, NGT] i32
        cnt = np.asarray(r["cnts"]).reshape(E).astype(np.int64)
        # slot (p, g) -> staged row g*128+p holds token permv[p, g]
        ids = permv.T.reshape(-1)                           # row-major g*128+p
        slot_in_e = np.tile(np.arange(CAPZ), E)
        valid = (ids < TPC) & (slot_in_e < cnt.repeat(CAPZ))
        sl_out = full[c * TPC : (c + 1) * TPC]
        sl_out[ids[valid]] = staged[valid]
    out = full.reshape(B, S, H)
    if run_kwargs:
        return out, res
    return out


# revision 29
# speedup vs baseline: 1.1247x; 1.1086x over previous
"""Distributed sparse MoE (top-1 routing) kernel for 8 TRN2 NeuronCores.

Strategy (zero-collective data-parallel):
  - Core c owns token slice [c*1024, (c+1)*1024) and ALL 8 expert weights
    (host-replicated bf16). No collectives -> core 0 never waits on the
    launch skew of its peers.
  - The sync queue is a dedicated streaming FIFO (HWDGE DMAs occupy their
    issuing engine for the whole transfer): router xT quarters first, then
    the 8 x 2MB expert weights (SBUF-resident), then the staged output
    writes. Small latency-critical DMAs go on the scalar queue.
  - Router: fp32 PE matmul in stream orientation (lhsT = router_w chunk,
    rhs = xT chunk) -> logitsT [8, 1024] in PSUM, bias via per-partition
    scalar add at eviction, then 8 small PE transposes give [128, 8]
    logit tiles (argmax must match the reference bit-for-bit: min top-2
    logit gap ~4e-5, so routing stays fp32 while expert GEMMs are bf16).
    Batched softmax: E=exp(logits) (|logit| <~ 6), per-8-group sum/max
    reductions, gate = max(E)/sum(E).
  - Selection packs id and gate into ONE value per token:
    v = (tokid + gate/2) after masking, compacted per expert by
    sparse_gather (capacity 256/expert). Compaction results stream to
    DRAM per expert and come back in ONE re-wrap read; on-chip unpack
    gives id = trunc(v), gate = 2*(v - trunc(v)). Pad slots are forced
    to the OOB sentinel AFTER the roundtrip (select, NaN-safe) using a
    count broadcast that overlaps the roundtrip itself.
  - Per expert: indirect-gather token rows (bf16; the SWDGE queue holds
    ONLY gathers), PE-transpose, bf16 GEMM vs resident W_e with fp32
    accumulate, bias + gate at PSUM eviction. Outputs are written
    CONTIGUOUSLY (direct DMA, line rate) as staged [2048, H] bf16 plus
    the 8KB permutation tensor; the host unpermutes (slot -> token row)
    while concatenating the 8 disjoint slice outputs and casting f32.
"""

import sys

sys.path.insert(0, "/opt/trn_rl_repo")

import ml_dtypes
import numpy as np

import concourse.bass as bass
import concourse.mybir as mybir
import concourse.tile as tile
from concourse import bacc
from concourse.bass_utils import run_bass_kernel_spmd
from concourse.masks import make_identity

F32 = mybir.dt.float32
BF16 = mybir.dt.bfloat16
I32 = mybir.dt.int32
U32 = mybir.dt.uint32

N_CORES = 8
B, S, H, E = 4, 2048, 1024, 8
T = B * S                # 8192 tokens
TPC = T // N_CORES       # 1024 tokens per core slice
TILES = TPC // 128       # 8 token tiles per slice
HC = H // 128            # 8 contraction chunks
CAPZ = 256               # per-(core,expert) token capacity (mean 128, sigma ~11)
ZTIL = CAPZ // 128       # 2 gathered token tiles per expert
NHALF = 2                # 1024 output dims in 2 x 512 psum halves
OOB = TPC                # out-of-bounds sentinel id (skipped / host-dropped)
SEL = TILES * E          # 64: free size of the [16, .] selection layout
NGT = E * ZTIL           # 16 gather tiles
PREF = 5                 # gather prefetch depth


def _body(tc, xt, xb, rw, rb, ew, eb, iota1, out, perm, cnts):
    nc = tc.nc
    P = 128
    Exp = mybir.ActivationFunctionType.Exp

    const = tc.alloc_tile_pool(name="const", bufs=1)

    # --- streaming FIFO (sync queue): xt chunks first, then weights ---
    xtp = tc.alloc_tile_pool(name="xtp", bufs=3)
    xcs = []
    for c in range(HC):
        xc = xtp.tile([P, TPC], F32, tag="xc")
        nc.sync.dma_start(xc[:], xt[c * P : (c + 1) * P, :])
        xcs.append(xc)
    w_sb = []
    for e in range(E):
        wt = const.tile([P, HC, H], BF16, name=f"w{e}")
        nc.sync.dma_start(
            wt[:], ew[e * H : (e + 1) * H, :].rearrange("(c p) d -> p c d", p=P)
        )
        w_sb.append(wt)

    # --- small constants (scalar queue) ---
    rw_sb = const.tile([P, HC, E], F32)
    nc.scalar.dma_start(rw_sb[:], rw.rearrange("(c p) e -> p c e", p=P))
    rb_sb = const.tile([E, 1], F32)
    nc.scalar.dma_start(rb_sb[:], rb[:])
    ident = const.tile([P, P], F32)
    make_identity(nc, ident)
    identb = const.tile([P, P], BF16)
    nc.vector.tensor_copy(identb[:], ident[:])
    iota_sb = const.tile([16, SEL], F32)
    nc.scalar.dma_start(iota_sb[:], iota1[:])

    dram = tc.alloc_tile_pool(name="dram", bufs=1, space="DRAM")
    dec_dram = dram.tile([P, 16], F32)
    ig_dram = dram.tile([E, CAPZ], F32)

    # ---- Phase A: router, stream orientation ----
    dec_sb = const.tile([P, 16], F32)
    lT_sb = const.tile([8, TPC], F32)
    logits = const.tile([P, TILES, E], F32)
    with tc.tile_pool(name="workA", bufs=2) as workA, tc.tile_pool(
        name="psumL", bufs=1, space="PSUM"
    ) as psumL, tc.tile_pool(name="psumR", bufs=1, space="PSUM") as psumR:
        lpT = psumL.tile([8, TPC], F32)
        for c in range(HC):
            for h in range(NHALF):
                nc.tensor.matmul(
                    lpT[:, h * 512 : (h + 1) * 512],
                    lhsT=rw_sb[:, c, :],
                    rhs=xcs[c][:, h * 512 : (h + 1) * 512],
                    start=(c == 0),
                    stop=(c == HC - 1),
                )
        # evict with router bias (per-partition scalar), then transpose
        nc.vector.tensor_scalar(
            lT_sb[:], lpT[:], rb_sb[:], None, op0=mybir.AluOpType.add
        )
        ptil = psumR.tile([P, TILES, E], F32)
        for t in range(TILES):
            nc.tensor.transpose(
                ptil[:, t, :], lT_sb[:, t * P : (t + 1) * P], ident[0:8, 0:8]
            )
        nc.vector.tensor_copy(
            logits[:].rearrange("p a b -> p (a b)"),
            ptil[:].rearrange("p a b -> p (a b)"),
        )
        # batched softmax pieces: exp, per-8-group sum and max
        expd = workA.tile([P, TILES, E], F32, tag="expd")
        nc.scalar.activation(
            expd[:].rearrange("p a b -> p (a b)"),
            logits[:].rearrange("p a b -> p (a b)"),
            Exp,
        )
        esum = workA.tile([P, TILES], F32, tag="esum")
        nc.vector.reduce_sum(esum[:], expd[:], mybir.AxisListType.X)
        emax = workA.tile([P, TILES], F32, tag="emax")
        nc.vector.reduce_max(emax[:], expd[:], mybir.AxisListType.X)
        erec = workA.tile([P, TILES], F32, tag="erec")
        nc.vector.reciprocal(erec[:], esum[:])
        nc.vector.tensor_tensor(
            dec_sb[:, 8:16], emax[:], erec[:], mybir.AluOpType.mult
        )
        for t in range(TILES):
            mx8 = workA.tile([P, 8], F32, tag="mx8")
            nc.vector.max(mx8[:], logits[:, t, :])
            mi = workA.tile([P, 8], U32, tag="mi")
            nc.vector.max_index(mi[:], mx8[:], logits[:, t, :])
            nc.vector.tensor_copy(dec_sb[:, t : t + 1], mi[:, 0:1])
    xtp.release()

    # ---- Phase B: selection — single-engine (gpsimd) chain, no cross-
    # engine ping-pong. The compaction output is pre-filled with the OOB
    # sentinel; sparse_gather only overwrites the slots it found, so tail
    # slots need no fixup (id -> 1024 skipped by bounds check / host).
    sel = tc.alloc_tile_pool(name="sel", bufs=1)
    stage_all = sel.tile([16, E, CAPZ // 16], F32)
    # roundtrip through DRAM to re-wrap [128,16] -> [16,128]
    nc.gpsimd.dma_start(dec_dram[:], dec_sb[:])
    dsb = sel.tile([16, 8, 16], F32)
    nc.gpsimd.dma_start(dsb[:], dec_dram[:].rearrange("(p a) c -> p a c", p=16))
    idx16 = sel.tile([16, SEL], F32)
    nc.vector.tensor_copy(idx16[:].rearrange("p (a b) -> p a b", a=8), dsb[:, :, 0:8])
    # packed compaction value: base = (tokid+1) + gate/2; the -1 of the
    # masking below shifts it to tokid + gate/2 for selected slots
    base = sel.tile([16, SEL], F32)
    nc.vector.tensor_scalar(
        base[:].rearrange("p (a b) -> p a b", a=8),
        dsb[:, :, 8:16],
        0.5,
        None,
        op0=mybir.AluOpType.mult,
    )
    nc.vector.tensor_tensor(base[:], base[:], iota_sb[:], mybir.AluOpType.add)
    val_all = sel.tile([16, E, SEL], F32)
    for e in range(E):
        eqv = val_all[:, e, :]
        nc.vector.tensor_scalar(
            eqv, idx16[:], float(e), None, op0=mybir.AluOpType.is_equal
        )
        nc.vector.tensor_tensor(eqv, base[:], eqv, mybir.AluOpType.mult)
        nc.vector.tensor_scalar_add(eqv, eqv, -1.0)
    cnt_all = sel.tile([1, E], U32)
    for e in range(E):
        nc.gpsimd.sparse_gather(
            stage_all[:, e, :], val_all[:, e, :], num_found=cnt_all[:, e : e + 1]
        )
    nc.gpsimd.dma_start(
        ig_dram[:].rearrange("e (f p) -> p e f", p=16), stage_all[:]
    )
    # ONE re-wrap read: [128, (e j)] per-partition slots, then unpack
    igp = sel.tile([P, NGT], F32)
    nc.gpsimd.dma_start(
        igp[:].rearrange("p (e j) -> p e j", e=E),
        ig_dram[:].rearrange("e (j p) -> p e j", p=P),
    )
    idsel = sel.tile([P, NGT], I32)
    nc.gpsimd.tensor_copy(idsel[:], igp[:])         # trunc to tokid
    # compaction tails hold garbage: send any id outside [0, TPC) to the
    # OOB sentinel so the gather skips it (int ops are NaN-free; negative
    # ids viewed as u32 are huge, so ONE unsigned compare catches both)
    okm = sel.tile([P, NGT], I32)
    nc.gpsimd.tensor_scalar(
        okm[:],
        idsel[:].bitcast(U32),
        TPC,
        None,
        op0=mybir.AluOpType.is_lt,
    )
    nc.gpsimd.tensor_scalar_add(idsel[:], idsel[:], -OOB)
    nc.gpsimd.tensor_tensor(idsel[:], idsel[:], okm[:], mybir.AluOpType.mult)
    nc.gpsimd.tensor_scalar_add(idsel[:], idsel[:], OOB)
    idxf = sel.tile([P, NGT], F32)
    nc.gpsimd.tensor_copy(idxf[:], idsel[:])
    gativ = sel.tile([P, NGT], F32)
    nc.gpsimd.tensor_tensor(gativ[:], igp[:], idxf[:], mybir.AluOpType.subtract)
    nc.gpsimd.tensor_scalar(gativ[:], gativ[:], 2.0, None, op0=mybir.AluOpType.mult)

    # ---- Phase C per expert: gather, transpose, GEMM, staged write ----
    with tc.tile_pool(name="ebp", bufs=2) as ebp, tc.tile_pool(
        name="workD", bufs=2
    ) as workD, tc.tile_pool(name="gathp", bufs=PREF) as gathp, tc.tile_pool(
        name="outp", bufs=3
    ) as outp, tc.tile_pool(name="psumT", bufs=3, space="PSUM") as psumT, tc.tile_pool(
        name="psumG", bufs=2, space="PSUM"
    ) as psumG:
        gtiles = {}

        def issue_gather(g):
            gt = gathp.tile([P, H], BF16, tag="gath")
            nc.gpsimd.indirect_dma_start(
                out=gt[:],
                out_offset=None,
                in_=xb[:],
                in_offset=bass.IndirectOffsetOnAxis(ap=idsel[:, g : g + 1], axis=0),
                bounds_check=TPC - 1,
                oob_is_err=False,
            )
            gtiles[g] = gt

        for g in range(PREF):
            issue_gather(g)
        for e in range(E):
            eb_sb = ebp.tile([1, H], F32, tag="eb_sb")
            nc.scalar.dma_start(eb_sb[:], eb[e : e + 1, :])
            b_rep = ebp.tile([P, H], F32, tag="b_rep")
            nc.gpsimd.partition_broadcast(b_rep[:], eb_sb[:])
            for j in range(ZTIL):
                g = e * ZTIL + j
                gath = gtiles.pop(g)
                xTg = workD.tile([P, HC, P], BF16, tag="xTg")
                pt = psumT.tile([P, H], BF16, tag="pt")
                for c in range(HC):
                    nc.tensor.transpose(
                        pt[:, c * P : (c + 1) * P],
                        gath[:, c * P : (c + 1) * P],
                        identb[:],
                    )
                if j % 2 == 0:
                    nc.scalar.copy(xTg[:].rearrange("p c d -> p (c d)"), pt[:])
                else:
                    nc.vector.tensor_copy(
                        xTg[:].rearrange("p c d -> p (c d)"), pt[:]
                    )
                outsb = outp.tile([P, H], BF16, tag="outsb")
                for h in range(NHALF):
                    pg = psumG.tile([P, 512], F32, tag="pg")
                    for c in range(HC):
                        nc.tensor.matmul(
                            pg[:],
                            lhsT=xTg[:, c, :],
                            rhs=w_sb[e][:, c, h * 512 : (h + 1) * 512],
                            start=(c == 0),
                            stop=(c == HC - 1),
                        )
                    nc.vector.tensor_tensor(
                        outsb[:, h * 512 : (h + 1) * 512],
                        pg[:],
                        b_rep[:, h * 512 : (h + 1) * 512],
                        mybir.AluOpType.add,
                    )
                    nc.vector.tensor_scalar_mul(
                        outsb[:, h * 512 : (h + 1) * 512],
                        outsb[:, h * 512 : (h + 1) * 512],
                        gativ[:, g : g + 1],
                    )
                nc.sync.dma_start(out[g * P : (g + 1) * P, :], outsb[:])
                if g + PREF < NGT:
                    issue_gather(g + PREF)
    nc.scalar.dma_start(perm[:], idsel[:])
    nc.scalar.dma_start(cnts[:], cnt_all[:])

    sel.release()
    dram.release()
    const.release()


def build_kernel():
    nc = bacc.Bacc(
        "TRN2",
        target_bir_lowering=False,
        debug=False,
        enable_asserts=True,
        num_devices=N_CORES,
    )
    xt = nc.dram_tensor("xt", [H, TPC], F32, kind="ExternalInput").ap()
    xb = nc.dram_tensor("xb", [TPC, H], BF16, kind="ExternalInput").ap()
    rw = nc.dram_tensor("router_w", [H, E], F32, kind="ExternalInput").ap()
    rb = nc.dram_tensor("router_b", [E, 1], F32, kind="ExternalInput").ap()
    ew = nc.dram_tensor("expert_w", [E * H, H], BF16, kind="ExternalInput").ap()
    eb = nc.dram_tensor("expert_b", [E, H], F32, kind="ExternalInput").ap()
    iota1 = nc.dram_tensor("iota1", [16, TILES * E], F32, kind="ExternalInput").ap()
    out = nc.dram_tensor("out", [E * CAPZ, H], BF16, kind="ExternalOutput").ap()
    perm = nc.dram_tensor("perm", [P128, NGT], I32, kind="ExternalOutput").ap()
    cnts = nc.dram_tensor("cnts", [1, E], U32, kind="ExternalOutput").ap()

    with tile.TileContext(nc) as tc:
        _body(tc, xt, xb, rw, rb, ew, eb, iota1, out, perm, cnts)
    nc.compile()
    return nc


P128 = 128
_CACHE = {}


def kernel(x, router_w, router_b, expert_w, expert_b, **run_kwargs):
    x = np.ascontiguousarray(np.asarray(x, dtype=np.float32))
    router_w = np.ascontiguousarray(np.asarray(router_w, dtype=np.float32))
    router_b = np.ascontiguousarray(np.asarray(router_b, dtype=np.float32))
    expert_w = np.ascontiguousarray(np.asarray(expert_w, dtype=np.float32))
    expert_b = np.ascontiguousarray(np.asarray(expert_b, dtype=np.float32))

    hs = x.reshape(T, H)
    ew_b = np.ascontiguousarray(
        expert_w.reshape(E * H, H).astype(ml_dtypes.bfloat16)
    )
    eb_r = np.ascontiguousarray(expert_b.reshape(E, H))
    rb_r = np.ascontiguousarray(router_b.reshape(E, 1))

    # iota1[p, j2]: local token id + 1 at selection position (p, j2)
    # j2 = jj*8 + col; token k = col*128 + 8*p + jj
    pp, j2 = np.meshgrid(np.arange(16), np.arange(TILES * E), indexing="ij")
    jj, col = j2 // 8, j2 % 8
    iota1 = (col * 128 + 8 * pp + jj + 1).astype(np.float32)

    if "nc" not in _CACHE:
        _CACHE["nc"] = build_kernel()
    nc = _CACHE["nc"]

    in_maps = []
    for c in range(N_CORES):
        sl = hs[c * TPC : (c + 1) * TPC]
        in_maps.append(
            {
                "xt": np.ascontiguousarray(sl.T),
                "xb": np.ascontiguousarray(sl.astype(ml_dtypes.bfloat16)),
                "router_w": router_w,
                "router_b": rb_r,
                "expert_w": ew_b,
                "expert_b": eb_r,
                "iota1": iota1,
            }
        )

    res = run_bass_kernel_spmd(nc, in_maps, core_ids=list(range(N_CORES)), **run_kwargs)
    full = np.empty((T, H), dtype=np.float32)
    for c, r in enumerate(res.results):
        staged = np.asarray(r["out"], dtype=np.float32)     # [E*CAPZ, H]
        permv = np.asarray(r["perm"])                       # [128, NGT] i32
        cnt = np.asarray(r["cnts"]).reshape(E).astype(np.int64)
        # slot (p, g) -> staged row g*128+p holds token permv[p, g]
        ids = permv.T.reshape(-1)                           # row-major g*128+p
        slot_in_e = np.tile(np.arange(CAPZ), E)
        valid = (ids < TPC) & (slot_in_e < cnt.repeat(CAPZ))
        sl_out = full[c * TPC : (c + 1) * TPC]
        sl_out[ids[valid]] = staged[valid]
    out = full.reshape(B, S, H)
    if run_kwargs:
        return out, res
    return out
